# revision 6
# baseline (speedup 1.0000x reference)
"""Trainium2 Bass kernel for nn_Encoder_5248450035714 (2-layer LSTM encoder).

x = emb[input_seq]; two LSTM layers; returns (h_n, c_n) each [2, B, H].
S=256, B=64, E=H=1024, vocab 32000.

Sharding: tensor-parallel over the 4H gate dim across 8 cores; core c owns
h-dims [128c, 128c+128). Per step the full h must be re-assembled on every
core. v2 replaces the per-step ncfw AllGather of h0 with direct SBUF->SBUF
remote_dma broadcasts (XOR-slot addressing: slot d of core r receives core
sigma(r,d)'s chunk; host-side weight permutation absorbs sigma, including
the measured XOR-2 on cross-die routes). h1 still rides the ncfw AllGather
(it has a full step of latency slack). Gate order (i,f,o,g) so one wide
sigmoid + one tanh cover the gate activations.
"""
import os
import sys

sys.path.insert(0, "/opt/trn_rl_repo")

import numpy as np
import ml_dtypes

BF16 = ml_dtypes.bfloat16

S, B, VOCAB, E, H = 256, 64, 32000, 1024, 1024
NCORES = 8
HC = H // NCORES          # 128 h-dims per core
KE = E // 128             # 8 contraction chunks over E/H
GATE_ORDER = (0, 1, 3, 2)  # i, f, o, g (block index into the 4H dim)

_CACHE = {}
_PREBUMP = []   # [(sem_num, sem_name, value)] applied in Tile scheduling sim


def _ensure_axon_hooks():
    try:
        import antenv
        if "/opt/trn_rl_repo/antenv" not in list(antenv.__path__):
            antenv.__path__.append("/opt/trn_rl_repo/antenv")
    except Exception:
        pass
    import types
    if "antenv.axon_hooks" not in sys.modules:
        mod = types.ModuleType("antenv.axon_hooks")
        hook = [None]

        def set_axon_ntff_profile_hook(h):
            hook[0] = h

        def get_axon_ntff_profile_hook():
            if hook[0] is None:
                try:
                    from trn_agent_boot.trn_boot import _ntff_profile_via_ctypes
                    hook[0] = _ntff_profile_via_ctypes(
                        "/opt/axon/libaxon_pjrt.so")
                except Exception:
                    return None
            return hook[0]

        mod.set_axon_ntff_profile_hook = set_axon_ntff_profile_hook
        mod.get_axon_ntff_profile_hook = get_axon_ntff_profile_hook
        sys.modules["antenv.axon_hooks"] = mod


def _install_coresim_patch():
    """Tile's scheduling sim is single-core; remote sem increments never
    arrive there. Pre-bump them (scheduling only — real waits still
    enforced on HW/MultiCoreSim)."""
    import concourse.tile as tile
    import concourse.bass_interp as bass_interp
    import concourse.mybir as mybir
    if getattr(tile, "_prebump_patched", False):
        return

    real = bass_interp.CoreSim

    class SchedCoreSim(real):
        def __init__(self, *a, **kw):
            super().__init__(*a, **kw)
            for num, name, val in _PREBUMP:
                self.update_semaphore(
                    mybir.SyncUpdate(
                        sync_type="semaphore", id=num, ant_name=name,
                        update_mode="sem-add-imm", update_value=val))

    tile.CoreSim = SchedCoreSim
    tile._prebump_patched = True


def sigma(r, d, d2d_xor=2):
    """Logical source core whose chunk lands in slot d of core r."""
    return r ^ d ^ (d2d_xor if d & 4 else 0)


def build_nc(n_steps=S, debug=False):
    import concourse.bacc as bacc
    import concourse.mybir as mybir
    import concourse.tile as tile
    from concourse.bass import _add_dep_helper

    _install_coresim_patch()
    _PREBUMP.clear()

    dt = mybir.dt
    AF = mybir.ActivationFunctionType
    nc = bacc.Bacc("TRN2", target_bir_lowering=False, debug=debug,
                   num_devices=NCORES)
    ntok = n_steps * B
    NT = min(512, ntok)   # token tile for proj0

    # ---- per-core inputs (host-sharded) ----
    tok = nc.dram_tensor("tok", [128, ntok // 16], dt.int16,
                         kind="ExternalInput")
    embc = nc.dram_tensor("embc", [VOCAB, 128], dt.bfloat16,
                          kind="ExternalInput")
    w_p0 = nc.dram_tensor("w_p0", [KE * 4 * 128, 128], dt.bfloat16,
                          kind="ExternalInput")   # proj0 lhsT tiles [k][m]
    w_r0 = nc.dram_tensor("w_r0", [KE * 4 * 128, 128], dt.bfloat16,
                          kind="ExternalInput")   # rec0 W_hh0^T, slot order
    w_r1 = nc.dram_tensor("w_r1", [2 * KE * 4 * 128, 128], dt.bfloat16,
                          kind="ExternalInput")   # [W_ih1 slot; W_hh1 nat]^T
    b0 = nc.dram_tensor("b0", [4 * 128, 1], dt.float32, kind="ExternalInput")
    b1b = nc.dram_tensor("b1b", [4 * 128, B], dt.float32,
                         kind="ExternalInput")    # b1 broadcast over batch

    out = nc.dram_tensor("out", [4 * 128, B], dt.float32,
                         kind="ExternalOutput")

    xp0 = nc.dram_tensor("xp0", [4 * 128, ntok], dt.float32, kind="Internal")

    rg = [list(range(NCORES))]

    with tile.TileContext(nc) as tc:
        with tc.tile_pool(name="dram", bufs=1, space="DRAM") as dram, \
             tc.tile_pool(name="wpool", bufs=1) as wpool, \
             tc.tile_pool(name="gather", bufs=1) as gpool, \
             tc.tile_pool(name="xtiles", bufs=2) as xpool, \
             tc.tile_pool(name="psum", bufs=1, space="PSUM") as psum_pool, \
             tc.tile_pool(name="ew", bufs=3) as ewpool, \
             tc.tile_pool(name="state", bufs=1) as spool:

            # rdma semaphores: per-(parity mod 3, sender-distance) arrival
            # sems + per-parity local drain sems
            rsA = [[None] + [nc.alloc_semaphore(f"rsA{q}_{d}")
                             for d in range(1, NCORES)] for q in range(3)]
            lsA = [nc.alloc_semaphore(f"lsA{q}") for q in range(3)]
            for q in range(3):
                for d in range(1, NCORES):
                    _PREBUMP.append((rsA[q][d].num, rsA[q][d].name, 1 << 20))
                _PREBUMP.append((lsA[q].num, lsA[q].name, 1 << 22))

            def dma_blocks(dst2d, src, nblk, f):
                """DRAM [(n p), f] -> SBUF [p, (n f)] block-row layout."""
                nc.sync.dma_start(
                    dst2d.rearrange("p (n f) -> p n f", f=f),
                    src.rearrange("(n p) f -> p n f", p=128))

            # ============ Phase 1: gather x^T e-chunk ============
            idx_sb = gpool.tile([128, ntok // 16], dt.int16, tag="idx")
            nc.sync.dma_start(idx_sb[:], tok[:])
            xt_mine = gpool.tile([128, 1, ntok], dt.bfloat16, tag="xt")
            GCH = min(256, ntok)  # idxs per gather instruction
            for gi in range(ntok // GCH):
                nc.gpsimd.dma_gather(
                    xt_mine[:, :, GCH * gi:GCH * (gi + 1)],
                    embc[:],
                    idx_sb[:, (GCH // 16) * gi:(GCH // 16) * (gi + 1)],
                    num_idxs=GCH, num_idxs_reg=GCH, elem_size=128,
                    transpose=True,
                )

            # ============ Phase 2: AllGather x^T ============
            ag_in = dram.tile([128, ntok], dt.bfloat16, tag="agin")
            xt_full = dram.tile([NCORES * 128, ntok], dt.bfloat16, tag="xtf",
                                addr_space="Shared")
            nc.sync.dma_start(ag_in[:], xt_mine[:, 0, :])
            nc.gpsimd.collective_compute(
                "AllGather", mybir.AluOpType.bypass,
                ins=[ag_in.opt()], outs=[xt_full.opt()], replica_groups=rg,
            )

            # ============ Phase 3: proj0 GEMM ============
            w0_sb = wpool.tile([128, KE * 4 * 128], dt.bfloat16, tag="w0")
            dma_blocks(w0_sb[:], w_p0[:], KE * 4, 128)
            b0_sb = wpool.tile([128, 4], dt.float32, tag="b0")
            dma_blocks(b0_sb[:], b0[:], 4, 1)
            for tt in range(ntok // NT):
                rhs = []
                for k in range(KE):
                    r = xpool.tile([128, NT], dt.bfloat16, tag=f"rhs{k}")
                    nc.sync.dma_start(
                        r[:], xt_full[128 * k:128 * (k + 1),
                                      NT * tt:NT * (tt + 1)])
                    rhs.append(r)
                for m in range(4):
                    ps = psum_pool.tile([128, NT], dt.float32, tag=f"ps{m}")
                    for k in range(KE):
                        proj0_last_mm = nc.tensor.matmul(
                            ps[:],
                            w0_sb[:, (k * 4 + m) * 128:(k * 4 + m + 1) * 128],
                            rhs[k][:],
                            start=(k == 0), stop=(k == KE - 1))
                    xo = ewpool.tile([128, NT], dt.float32, tag="xo")
                    nc.scalar.activation(xo[:], ps[:], AF.Identity,
                                         bias=b0_sb[:, m:m + 1])
                    nc.sync.dma_start(
                        xp0[128 * m:128 * (m + 1), NT * tt:NT * (tt + 1)],
                        xo[:])

            # ============ Phases 4+5: recurrences ============
            w0r_sb = wpool.tile([128, KE * 4 * 128], dt.bfloat16, tag="w0r")
            dma_blocks(w0r_sb[:], w_r0[:], KE * 4, 128)
            w1r_sb = wpool.tile([128, 2 * KE * 4 * 128], dt.bfloat16,
                                tag="w1r")
            dma_blocks(w1r_sb[:], w_r1[:], 2 * KE * 4, 128)
            b1_sb = spool.tile([128, 4 * B], dt.float32, tag="b1")
            dma_blocks(b1_sb[:], b1b[:], 4, B)

            # persistent state
            c0_sb = spool.tile([128, B], dt.float32, tag="c0")
            c1_sb = spool.tile([128, B], dt.float32, tag="c1")
            nc.vector.memset(c0_sb[:], 0.0)
            nc.vector.memset(c1_sb[:], 0.0)

            # h0 gather buffers (slot d cols [64d,64d+64)), mod-3 rotation
            gathA = [spool.tile([128, NCORES * B], dt.bfloat16,
                                tag=f"gA{q}", name=f"gA{q}")
                     for q in range(3)]
            # h1 path (ncfw AG): bounce + per-step shared tiles + readback
            h1c = [spool.tile([128, B], dt.bfloat16, tag=f"h1c{p}",
                              name=f"h1c{p}") for p in range(2)]
            h1_buf = [spool.tile([128, KE * B], dt.bfloat16,
                                 tag=f"h1b{i}", name=f"h1b{i}")
                      for i in range(2)]
            h1ag = [dram.tile([NCORES * 128, B], dt.bfloat16,
                              tag=f"h1ag_{t}", name=f"h1ag_{t}",
                              addr_space="Shared")
                    for t in range(max(n_steps - 1, 1))]
            agb1 = dram.tile([2, 128, B], dt.bfloat16, tag="agb1")

            sig, tnh = AF.Sigmoid, AF.Tanh
            B3 = 3 * B

            def cell(layer, t, ps, addend, h_out):
                """LSTM cell elementwise: gates=(ps+addend) -> h_out (bf16).
                Gate cols: [i f o | g]. Returns the h-mul instruction."""
                c_sb = c0_sb if layer == 0 else c1_sb
                g_sb = ewpool.tile([128, 4 * B], dt.float32,
                                   tag=f"g{layer}")
                if ps is None:
                    nc.scalar.activation(g_sb[:, 0:B3], addend[:, 0:B3], sig)
                    nc.scalar.activation(g_sb[:, B3:], addend[:, B3:], tnh)
                else:
                    nc.vector.tensor_add(g_sb[:], ps[:], addend[:])
                    nc.scalar.activation(g_sb[:, 0:B3], g_sb[:, 0:B3], sig)
                    nc.scalar.activation(g_sb[:, B3:], g_sb[:, B3:], tnh)
                ig = ewpool.tile([128, B], dt.float32, tag=f"ig{layer}")
                nc.vector.tensor_mul(ig[:], g_sb[:, 0:B], g_sb[:, B3:])
                fc = ewpool.tile([128, B], dt.float32, tag=f"fc{layer}")
                nc.vector.tensor_mul(fc[:], g_sb[:, B:2 * B], c_sb[:])
                nc.vector.tensor_add(c_sb[:], ig[:], fc[:])
                tc_sb = ewpool.tile([128, B], dt.float32, tag=f"tc{layer}")
                nc.scalar.activation(tc_sb[:], c_sb[:], tnh)
                return nc.vector.tensor_mul(h_out, g_sb[:, 2 * B:B3], tc_sb[:])

            last_mm_prev = proj0_last_mm  # anchor for the first PE waits
            prev_trig = None
            trig_hist = {}           # t -> trigger instr
            l1_last_mm = {}          # t1 -> last L1 matmul instr
            h1_mul_final = None

            for t in range(n_steps + 1):
                # ---- arrival waits for gather(t-1) ----
                waits = []
                if t >= 1:
                    q = (t - 1) % 3
                    thr = 2 * ((t - 1) // 3 + 1)
                    for d in range(1, NCORES):
                        w = nc.tensor.wait_ge(rsA[q][d], thr)
                        if last_mm_prev is not None:
                            _add_dep_helper(w.ins, last_mm_prev.ins, True,
                                            "anchor wait after prev window")
                        waits.append(w)

                last_mm = None
                # ---- L0(t) matmuls ----
                ps0 = None
                if t < n_steps and t >= 1:
                    gin = gathA[(t - 1) % 3]
                    ps0 = psum_pool.tile([128, 4 * B], dt.float32,
                                         tag=f"gps0{t % 2}")
                    for m in range(4):
                        for k in range(KE):
                            mm = nc.tensor.matmul(
                                ps0[:, B * m:B * (m + 1)],
                                w0r_sb[:, (k * 4 + m) * 128:
                                       (k * 4 + m + 1) * 128],
                                gin[:, B * k:B * (k + 1)],
                                start=(k == 0), stop=(k == KE - 1))
                            if k == 0:
                                for w in waits:
                                    _add_dep_helper(mm.ins, w.ins, True,
                                                    "gather arrival")
                            last_mm = mm

                # ---- L1(t-1) matmuls ----
                ps1 = None
                if t >= 1:
                    t1 = t - 1
                    gin = gathA[t1 % 3]
                    hb = h1_buf[t1 % 2]
                    nk = KE if t1 == 0 else 2 * KE
                    ps1 = psum_pool.tile([128, 4 * B], dt.float32,
                                         tag=f"gps1{t % 2}")
                    for m in range(4):
                        for k in range(nk):
                            if k < KE:
                                rhs_ap = gin[:, B * k:B * (k + 1)]
                                wofs = (k * 4 + m) * 128
                            else:
                                rhs_ap = hb[:, B * (k - KE):B * (k - KE + 1)]
                                wofs = (k * 4 + m) * 128
                            mm = nc.tensor.matmul(
                                ps1[:, B * m:B * (m + 1)],
                                w1r_sb[:, wofs:wofs + 128],
                                rhs_ap,
                                start=(k == 0), stop=(k == nk - 1))
                            if k == 0:
                                for w in waits:
                                    _add_dep_helper(mm.ins, w.ins, True,
                                                    "gather arrival")
                            last_mm = mm
                    l1_last_mm[t1] = last_mm

                # ---- L0 cell + rdma sends ----
                if t < n_steps:
                    qs = t % 3
                    xp_sb = ewpool.tile([128, 4 * B], dt.float32,
                                        tag=f"xp{t % 2}")
                    dma_blocks(xp_sb[:], xp0[:, B * t:B * (t + 1)], 4, B)
                    h_slot0 = gathA[qs][:, 0:B]
                    hmul = cell(0, t, ps0, xp_sb, h_slot0)
                    if t >= 3:
                        wl = nc.vector.wait_ge(lsA[qs], 112 * (t // 3))
                        _add_dep_helper(wl.ins, trig_hist[t - 1].ins, True,
                                        "anchor lsem wait")
                        _add_dep_helper(hmul.ins, wl.ins, True, "src drain")
                    # 7 broadcasts of my slot-0 chunk
                    for d in range(1, NCORES):
                        rdests = [None] * NCORES
                        rdests[d] = (0, d)
                        bc = nc.gpsimd.remote_dma_broadcast(
                            gathA[qs][:, B * d:B * (d + 1)],
                            h_slot0,
                            rsA[qs][d], lsA[qs], rdests=rdests)
                        if d == 1 and prev_trig is not None:
                            _add_dep_helper(bc.ins, prev_trig.ins, True,
                                            "pool round order")
                    trig = nc.gpsimd.trigger_dma(count=7)
                    if t - 2 >= 0 and (t - 2) in l1_last_mm:
                        _add_dep_helper(trig.ins, l1_last_mm[t - 2].ins, True,
                                        "protect L1 reads before overwrite")
                    trig_hist[t] = trig
                    prev_trig = trig

                # ---- L1 cell + ncfw AG ----
                if t >= 1:
                    t1 = t - 1
                    h1mul = cell(1, t1, ps1, b1_sb, h1c[t1 % 2][:])
                    if t1 < n_steps - 1:
                        nc.sync.dma_start(agb1[t1 % 2, :, :], h1c[t1 % 2][:])
                        nc.gpsimd.collective_compute(
                            "AllGather", mybir.AluOpType.bypass,
                            ins=[agb1[t1 % 2, :, :].opt()],
                            outs=[h1ag[t1].opt()], replica_groups=rg)
                        dma_blocks(h1_buf[(t1 + 1) % 2][:], h1ag[t1][:],
                                   KE, B)
                    else:
                        h1_mul_final = h1mul

                if last_mm is not None:
                    last_mm_prev = last_mm

            # ---- outputs ----
            of = ewpool.tile([128, B], dt.float32, tag="of")
            cp0 = nc.scalar.activation(of[:], gathA[(n_steps - 1) % 3][:, 0:B],
                                       AF.Copy)
            nc.sync.dma_start(out[0:128, :], of[:])
            nc.sync.dma_start(out[128:256, :], c0_sb[:])
            of2 = ewpool.tile([128, B], dt.float32, tag="of2")
            nc.scalar.activation(of2[:], h1c[(n_steps - 1) % 2][:], AF.Copy)
            nc.sync.dma_start(out[256:384, :], of2[:])
            nc.sync.dma_start(out[384:512, :], c1_sb[:])

    nc.compile()
    return nc


def _host_prep(inputs, n_steps=S, d2d_xor=2):
    """Build per-core in_maps from full inputs."""
    seq = np.asarray(inputs["input_seq"])[:n_steps].astype(np.int64)
    emb = np.asarray(inputs["emb"], dtype=np.float32)
    ntok = n_steps * B

    toks = seq.reshape(-1).astype(np.int16)  # vocab < 32768
    wrapped = toks.reshape(ntok // 16, 16).T.copy()       # [16, ntok/16]
    wrapped128 = np.tile(wrapped, (8, 1)).astype(np.int16)  # [128, ntok/16]

    w_ih_0T = np.asarray(inputs["w_ih_0"], np.float32).T
    w_hh_0T = np.asarray(inputs["w_hh_0"], np.float32).T
    w_ih_1T = np.asarray(inputs["w_ih_1"], np.float32).T
    w_hh_1T = np.asarray(inputs["w_hh_1"], np.float32).T
    b0sum = (np.asarray(inputs["b_ih_0"], np.float32) +
             np.asarray(inputs["b_hh_0"], np.float32))
    b1sum = (np.asarray(inputs["b_ih_1"], np.float32) +
             np.asarray(inputs["b_hh_1"], np.float32))

    in_maps = []
    for c in range(NCORES):
        m = {"tok": wrapped128,
             "embc": emb[:, 128 * c:128 * (c + 1)].astype(BF16)}

        def tiles(wT, nk, kperm=None):
            """lhsT tiles [(k*4+m)*128, 128]; cols = this core's 512 gates in
            GATE_ORDER; row-chunk k taken from contraction chunk kperm[k]."""
            cols = np.concatenate(
                [wT[:, H * gb + HC * c: H * gb + HC * (c + 1)]
                 for gb in GATE_ORDER], axis=1)  # [K, 512]
            arr = np.zeros((nk * 4 * 128, 128), dtype=BF16)
            for k in range(nk):
                ks = k if kperm is None else kperm[k]
                for mm in range(4):
                    arr[(k * 4 + mm) * 128:(k * 4 + mm + 1) * 128] = \
                        cols[128 * ks:128 * (ks + 1),
                             128 * mm:128 * (mm + 1)].astype(BF16)
            return arr

        slot_perm = [sigma(c, d, d2d_xor) for d in range(NCORES)]
        m["w_p0"] = tiles(w_ih_0T, KE)
        m["w_r0"] = tiles(w_hh_0T, KE, kperm=slot_perm)

        w1 = np.zeros((2 * KE * 4 * 128, 128), dtype=BF16)
        w1[:KE * 4 * 128] = tiles(w_ih_1T, KE, kperm=slot_perm)
        w1[KE * 4 * 128:] = tiles(w_hh_1T, KE)
        m["w_r1"] = w1

        def bias(bsum):
            v = np.concatenate(
                [bsum[H * gb + HC * c: H * gb + HC * (c + 1)]
                 for gb in GATE_ORDER])
            return v.reshape(4 * 128, 1).astype(np.float32)

        m["b0"] = bias(b0sum)
        m["b1b"] = np.tile(bias(b1sum), (1, B)).astype(np.float32)
        in_maps.append(m)
    return in_maps


def _assemble(results):
    h_n = np.zeros((2, B, H), np.float32)
    c_n = np.zeros((2, B, H), np.float32)
    for c in range(NCORES):
        o = results[c]["out"]
        h_n[0][:, HC * c:HC * (c + 1)] = o[0:128].T
        c_n[0][:, HC * c:HC * (c + 1)] = o[128:256].T
        h_n[1][:, HC * c:HC * (c + 1)] = o[256:384].T
        c_n[1][:, HC * c:HC * (c + 1)] = o[384:512].T
    return h_n, c_n


def run_on_hw(inputs, n_steps=S, trace=False):
    _ensure_axon_hooks()
    from concourse.bass_utils import run_bass_kernel_spmd
    if n_steps not in _CACHE:
        _CACHE[n_steps] = build_nc(n_steps)
    nc = _CACHE[n_steps]
    in_maps = _host_prep(inputs, n_steps, d2d_xor=2)
    res = run_bass_kernel_spmd(nc, in_maps, core_ids=list(range(NCORES)),
                               trace=trace)
    h_n, c_n = _assemble(res.results)
    return (h_n, c_n), res


def kernel(**inputs):
    (h_n, c_n), _ = run_on_hw(inputs, S, trace=False)
    return (h_n, c_n)


if __name__ == "__main__":
    ns = int(os.environ.get("NSTEPS", "4"))
    build_nc(ns)
    print("build OK", ns)


# revision 9
# speedup vs baseline: 1.0658x; 1.0658x over previous
"""Trainium2 Bass kernel for nn_Encoder_5248450035714 (2-layer LSTM encoder).

x = emb[input_seq]; two LSTM layers; returns (h_n, c_n) each [2, B, H].
S=256, B=64, E=H=1024, vocab 32000.

Sharding: tensor-parallel over the 4H gate dim across 8 cores; core c owns
h-dims [128c, 128c+128). Per step every core needs the full h of both
layers; v3 assembles it with direct SBUF->SBUF remote_dma broadcasts
(no ncfw collective in the loop): one message per step carrying
[h0(t) | h1(t-1)] in XOR-slot addressing — slot d of core r holds core
sigma(r,d)'s chunk; the host-side weight permutation absorbs sigma,
including the measured extra XOR-2 on cross-die routes. Gate order
(i,f,o,g) so one wide sigmoid + one tanh cover the gate activations.
"""
import os
import sys

sys.path.insert(0, "/opt/trn_rl_repo")

import numpy as np
import ml_dtypes

BF16 = ml_dtypes.bfloat16

S, B, VOCAB, E, H = 256, 64, 32000, 1024, 1024
NCORES = 8
HC = H // NCORES          # 128 h-dims per core
KE = E // 128             # 8 contraction chunks over E/H
SW = 2 * B                # slot width in gathA: [h0 64 | h1 64]
GATE_ORDER = (0, 1, 3, 2)  # i, f, o, g (block index into the 4H dim)

_CACHE = {}
_PREBUMP = []   # [(sem_num, sem_name, value)] applied in Tile scheduling sim


def _ensure_axon_hooks():
    try:
        import antenv
        if "/opt/trn_rl_repo/antenv" not in list(antenv.__path__):
            antenv.__path__.append("/opt/trn_rl_repo/antenv")
    except Exception:
        pass
    import types
    if "antenv.axon_hooks" not in sys.modules:
        mod = types.ModuleType("antenv.axon_hooks")
        hook = [None]

        def set_axon_ntff_profile_hook(h):
            hook[0] = h

        def get_axon_ntff_profile_hook():
            if hook[0] is None:
                try:
                    from trn_agent_boot.trn_boot import _ntff_profile_via_ctypes
                    hook[0] = _ntff_profile_via_ctypes(
                        "/opt/axon/libaxon_pjrt.so")
                except Exception:
                    return None
            return hook[0]

        mod.set_axon_ntff_profile_hook = set_axon_ntff_profile_hook
        mod.get_axon_ntff_profile_hook = get_axon_ntff_profile_hook
        sys.modules["antenv.axon_hooks"] = mod


def _install_coresim_patch():
    """Tile's scheduling sim is single-core; remote sem increments never
    arrive there. Pre-bump them (scheduling only — real waits still
    enforced on HW/MultiCoreSim)."""
    import concourse.tile as tile
    import concourse.bass_interp as bass_interp
    import concourse.mybir as mybir
    if getattr(tile, "_prebump_patched", False):
        return

    real = bass_interp.CoreSim

    class SchedCoreSim(real):
        def __init__(self, *a, **kw):
            super().__init__(*a, **kw)
            for num, name, val in _PREBUMP:
                self.update_semaphore(
                    mybir.SyncUpdate(
                        sync_type="semaphore", id=num, ant_name=name,
                        update_mode="sem-add-imm", update_value=val))

    tile.CoreSim = SchedCoreSim
    tile._prebump_patched = True


def sigma(r, d, d2d_xor=2):
    """Logical source core whose chunk lands in slot d of core r."""
    return r ^ d ^ (d2d_xor if d & 4 else 0)


def build_nc(n_steps=S, debug=False):
    import concourse.bacc as bacc
    import concourse.mybir as mybir
    import concourse.tile as tile
    from concourse.bass import _add_dep_helper

    _install_coresim_patch()
    _PREBUMP.clear()

    dt = mybir.dt
    AF = mybir.ActivationFunctionType
    nc = bacc.Bacc("TRN2", target_bir_lowering=False, debug=debug,
                   num_devices=NCORES)
    ntok = n_steps * B
    NT = min(512, ntok)   # token tile for proj0

    # ---- per-core inputs (host-sharded) ----
    tok = nc.dram_tensor("tok", [128, ntok // 16], dt.int16,
                         kind="ExternalInput")
    embc = nc.dram_tensor("embc", [VOCAB, 128], dt.bfloat16,
                          kind="ExternalInput")
    w_p0 = nc.dram_tensor("w_p0", [KE * 4 * 128, 128], dt.bfloat16,
                          kind="ExternalInput")   # proj0 lhsT tiles [k][m]
    w_r0 = nc.dram_tensor("w_r0", [KE * 4 * 128, 128], dt.bfloat16,
                          kind="ExternalInput")   # rec0 W_hh0^T, slot order
    w_r1 = nc.dram_tensor("w_r1", [2 * KE * 4 * 128, 128], dt.bfloat16,
                          kind="ExternalInput")   # [W_ih1; W_hh1]^T slot order
    b0 = nc.dram_tensor("b0", [4 * 128, 1], dt.float32, kind="ExternalInput")
    b1b = nc.dram_tensor("b1b", [4 * 128, B], dt.float32,
                         kind="ExternalInput")    # b1 broadcast over batch

    out = nc.dram_tensor("out", [4 * 128, B], dt.float32,
                         kind="ExternalOutput")

    xp0 = nc.dram_tensor("xp0", [4 * 128, ntok], dt.float32, kind="Internal")

    rg = [list(range(NCORES))]

    with tile.TileContext(nc) as tc:
        with tc.tile_pool(name="dram", bufs=1, space="DRAM") as dram, \
             tc.tile_pool(name="wpool", bufs=1) as wpool, \
             tc.tile_pool(name="gather", bufs=1) as gpool, \
             tc.tile_pool(name="xtiles", bufs=2) as xpool, \
             tc.tile_pool(name="psum", bufs=1, space="PSUM") as psum_pool, \
             tc.tile_pool(name="ew", bufs=3) as ewpool, \
             tc.tile_pool(name="state", bufs=1) as spool:

            # rdma semaphores: per-(parity mod 3, sender-distance) arrival
            # sems + per-parity local drain sems
            rsA = [[None] + [nc.alloc_semaphore(f"rsA{q}_{d}")
                             for d in range(1, NCORES)] for q in range(3)]
            lsA = [nc.alloc_semaphore(f"lsA{q}") for q in range(3)]
            for q in range(3):
                for d in range(1, NCORES):
                    _PREBUMP.append((rsA[q][d].num, rsA[q][d].name, 1 << 20))
                _PREBUMP.append((lsA[q].num, lsA[q].name, 1 << 22))

            def dma_blocks(dst2d, src, nblk, f):
                """DRAM [(n p), f] -> SBUF [p, (n f)] block-row layout."""
                nc.sync.dma_start(
                    dst2d.rearrange("p (n f) -> p n f", f=f),
                    src.rearrange("(n p) f -> p n f", p=128))

            # ============ Phase 1: gather x^T e-chunk ============
            idx_sb = gpool.tile([128, ntok // 16], dt.int16, tag="idx")
            nc.sync.dma_start(idx_sb[:], tok[:])
            xt_mine = gpool.tile([128, 1, ntok], dt.bfloat16, tag="xt")
            GCH = min(256, ntok)  # idxs per gather instruction
            for gi in range(ntok // GCH):
                nc.gpsimd.dma_gather(
                    xt_mine[:, :, GCH * gi:GCH * (gi + 1)],
                    embc[:],
                    idx_sb[:, (GCH // 16) * gi:(GCH // 16) * (gi + 1)],
                    num_idxs=GCH, num_idxs_reg=GCH, elem_size=128,
                    transpose=True,
                )

            # ============ Phase 2: AllGather x^T ============
            ag_in = dram.tile([128, ntok], dt.bfloat16, tag="agin")
            xt_full = dram.tile([NCORES * 128, ntok], dt.bfloat16, tag="xtf",
                                addr_space="Shared")
            nc.sync.dma_start(ag_in[:], xt_mine[:, 0, :])
            nc.gpsimd.collective_compute(
                "AllGather", mybir.AluOpType.bypass,
                ins=[ag_in.opt()], outs=[xt_full.opt()], replica_groups=rg,
            )

            # ============ Phase 3: proj0 GEMM ============
            w0_sb = wpool.tile([128, KE * 4 * 128], dt.bfloat16, tag="w0")
            dma_blocks(w0_sb[:], w_p0[:], KE * 4, 128)
            b0_sb = wpool.tile([128, 4], dt.float32, tag="b0")
            dma_blocks(b0_sb[:], b0[:], 4, 1)
            proj0_last_mm = None
            for tt in range(ntok // NT):
                rhs = []
                for k in range(KE):
                    r = xpool.tile([128, NT], dt.bfloat16, tag=f"rhs{k}")
                    nc.sync.dma_start(
                        r[:], xt_full[128 * k:128 * (k + 1),
                                      NT * tt:NT * (tt + 1)])
                    rhs.append(r)
                for m in range(4):
                    ps = psum_pool.tile([128, NT], dt.float32, tag=f"ps{m}")
                    for k in range(KE):
                        proj0_last_mm = nc.tensor.matmul(
                            ps[:],
                            w0_sb[:, (k * 4 + m) * 128:(k * 4 + m + 1) * 128],
                            rhs[k][:],
                            start=(k == 0), stop=(k == KE - 1))
                    xo = ewpool.tile([128, NT], dt.float32, tag="xo")
                    nc.scalar.activation(xo[:], ps[:], AF.Identity,
                                         bias=b0_sb[:, m:m + 1])
                    nc.sync.dma_start(
                        xp0[128 * m:128 * (m + 1), NT * tt:NT * (tt + 1)],
                        xo[:])

            # ============ Phases 4+5: recurrences ============
            w0r_sb = wpool.tile([128, KE * 4 * 128], dt.bfloat16, tag="w0r")
            dma_blocks(w0r_sb[:], w_r0[:], KE * 4, 128)
            w1r_sb = wpool.tile([128, 2 * KE * 4 * 128], dt.bfloat16,
                                tag="w1r")
            dma_blocks(w1r_sb[:], w_r1[:], 2 * KE * 4, 128)
            b1_sb = spool.tile([128, 4 * B], dt.float32, tag="b1")
            dma_blocks(b1_sb[:], b1b[:], 4, B)

            # persistent state
            c0_sb = spool.tile([128, B], dt.float32, tag="c0")
            c1_sb = spool.tile([128, B], dt.float32, tag="c1")
            nc.vector.memset(c0_sb[:], 0.0)
            nc.vector.memset(c1_sb[:], 0.0)

            # gather buffers: slot d cols [SW*d, SW*d+SW) = [h0 | h1] of
            # core sigma(r,d); mod-3 rotation
            gathA = [spool.tile([128, NCORES * SW], dt.bfloat16,
                                tag=f"gA{q}", name=f"gA{q}")
                     for q in range(3)]
            # h1-half of message(0) is h1(-1) = 0 (sent before any h1 exists)
            nc.vector.memset(gathA[0][:, B:2 * B], 0.0)

            sig, tnh = AF.Sigmoid, AF.Tanh
            B3 = 3 * B

            def cell(layer, ps, addend, h_out):
                """LSTM cell elementwise: gates=(ps+addend) -> h_out (bf16).
                Gate cols: [i f o | g]. Returns the h-mul instruction."""
                c_sb = c0_sb if layer == 0 else c1_sb
                g_sb = ewpool.tile([128, 4 * B], dt.float32,
                                   tag=f"g{layer}")
                if ps is None:
                    nc.scalar.activation(g_sb[:, 0:B3], addend[:, 0:B3], sig)
                    nc.scalar.activation(g_sb[:, B3:], addend[:, B3:], tnh)
                else:
                    nc.vector.tensor_add(g_sb[:], ps[:], addend[:])
                    nc.scalar.activation(g_sb[:, 0:B3], g_sb[:, 0:B3], sig)
                    nc.scalar.activation(g_sb[:, B3:], g_sb[:, B3:], tnh)
                ig = ewpool.tile([128, B], dt.float32, tag=f"ig{layer}")
                nc.vector.tensor_mul(ig[:], g_sb[:, 0:B], g_sb[:, B3:])
                fc = ewpool.tile([128, B], dt.float32, tag=f"fc{layer}")
                nc.vector.tensor_mul(fc[:], g_sb[:, B:2 * B], c_sb[:])
                nc.vector.tensor_add(c_sb[:], ig[:], fc[:])
                tc_sb = ewpool.tile([128, B], dt.float32, tag=f"tc{layer}")
                nc.scalar.activation(tc_sb[:], c_sb[:], tnh)
                return nc.vector.tensor_mul(h_out, g_sb[:, 2 * B:B3],
                                            tc_sb[:])

            last_mm_prev = proj0_last_mm  # anchor for the first PE waits
            prev_trig = None
            trig_hist = {}           # t -> trigger instr
            l1_last_mm = {}          # t1 -> last L1 matmul instr
            h1_mul = {}              # t1 -> h1 cell output instr

            for t in range(n_steps + 1):
                # ---- arrival waits for gather(t-1) ----
                waits = []
                if t >= 1:
                    q = (t - 1) % 3
                    thr = 2 * ((t - 1) // 3 + 1)
                    for d in range(1, NCORES):
                        w = nc.tensor.wait_ge(rsA[q][d], thr)
                        if last_mm_prev is not None:
                            _add_dep_helper(w.ins, last_mm_prev.ins, True,
                                            "anchor wait after prev window")
                        waits.append(w)

                last_mm = None
                # ---- L0(t) matmuls ----
                ps0 = None
                if t < n_steps and t >= 1:
                    gin = gathA[(t - 1) % 3]
                    ps0 = psum_pool.tile([128, 4 * B], dt.float32,
                                         tag=f"gps0{t % 2}")
                    for m in range(4):
                        for k in range(KE):
                            mm = nc.tensor.matmul(
                                ps0[:, B * m:B * (m + 1)],
                                w0r_sb[:, (k * 4 + m) * 128:
                                       (k * 4 + m + 1) * 128],
                                gin[:, SW * k:SW * k + B],
                                start=(k == 0), stop=(k == KE - 1))
                            if k == 0:
                                for w in waits:
                                    _add_dep_helper(mm.ins, w.ins, True,
                                                    "gather arrival")
                            last_mm = mm

                # ---- L1(t-1) matmuls ----
                ps1 = None
                if t >= 1:
                    t1 = t - 1
                    gin = gathA[t1 % 3]
                    nk = KE if t1 == 0 else 2 * KE
                    ps1 = psum_pool.tile([128, 4 * B], dt.float32,
                                         tag=f"gps1{t % 2}")
                    for m in range(4):
                        for k in range(nk):
                            if k < KE:   # ys0(t1) = gathered h0, slot k
                                rhs_ap = gin[:, SW * k:SW * k + B]
                            else:        # h1(t1-1), slot k-KE, second half
                                kk = k - KE
                                rhs_ap = gin[:, SW * kk + B:SW * kk + 2 * B]
                            wofs = (k * 4 + m) * 128
                            mm = nc.tensor.matmul(
                                ps1[:, B * m:B * (m + 1)],
                                w1r_sb[:, wofs:wofs + 128],
                                rhs_ap,
                                start=(k == 0), stop=(k == nk - 1))
                            if k == 0:
                                for w in waits:
                                    _add_dep_helper(mm.ins, w.ins, True,
                                                    "gather arrival")
                            last_mm = mm
                    l1_last_mm[t1] = last_mm

                # ---- cells into the message slot (slot 0 of gathA[t%3]) ----
                qs = t % 3
                wl = None
                if t >= 3:
                    wl = nc.vector.wait_ge(lsA[qs], 112 * (t // 3))
                    _add_dep_helper(wl.ins, trig_hist[t - 1].ins, True,
                                    "anchor lsem wait")
                if t < n_steps:
                    xp_sb = ewpool.tile([128, 4 * B], dt.float32,
                                        tag=f"xp{t % 2}")
                    dma_blocks(xp_sb[:], xp0[:, B * t:B * (t + 1)], 4, B)
                    hmul0 = cell(0, ps0, xp_sb, gathA[qs][:, 0:B])
                    if wl is not None:
                        _add_dep_helper(hmul0.ins, wl.ins, True, "src drain")
                if t >= 1:
                    hmul1 = cell(1, ps1, b1_sb, gathA[qs][:, B:2 * B])
                    if wl is not None:
                        _add_dep_helper(hmul1.ins, wl.ins, True, "src drain")
                    h1_mul[t - 1] = hmul1

                # ---- rdma broadcasts of the combined message ----
                if t < n_steps:
                    for d in range(1, NCORES):
                        rdests = [None] * NCORES
                        rdests[d] = (0, d)
                        bc = nc.gpsimd.remote_dma_broadcast(
                            gathA[qs][:, SW * d:SW * (d + 1)],
                            gathA[qs][:, 0:SW],
                            rsA[qs][d], lsA[qs], rdests=rdests)
                        if d == 1 and prev_trig is not None:
                            _add_dep_helper(bc.ins, prev_trig.ins, True,
                                            "pool round order")
                    trig = nc.gpsimd.trigger_dma(count=7)
                    if (t - 2) in l1_last_mm:
                        _add_dep_helper(trig.ins, l1_last_mm[t - 2].ins, True,
                                        "protect L1 reads before overwrite")
                    trig_hist[t] = trig
                    prev_trig = trig

                if last_mm is not None:
                    last_mm_prev = last_mm

            # ---- outputs ----
            of = ewpool.tile([128, B], dt.float32, tag="of")
            nc.scalar.activation(of[:], gathA[(n_steps - 1) % 3][:, 0:B],
                                 AF.Copy)
            nc.sync.dma_start(out[0:128, :], of[:])
            nc.sync.dma_start(out[128:256, :], c0_sb[:])
            of2 = ewpool.tile([128, B], dt.float32, tag="of2")
            nc.scalar.activation(of2[:], gathA[n_steps % 3][:, B:2 * B],
                                 AF.Copy)
            nc.sync.dma_start(out[256:384, :], of2[:])
            nc.sync.dma_start(out[384:512, :], c1_sb[:])

    nc.compile()
    return nc


def _host_prep(inputs, n_steps=S, d2d_xor=2):
    """Build per-core in_maps from full inputs."""
    seq = np.asarray(inputs["input_seq"])[:n_steps].astype(np.int64)
    emb = np.asarray(inputs["emb"], dtype=np.float32)
    ntok = n_steps * B

    toks = seq.reshape(-1).astype(np.int16)  # vocab < 32768
    wrapped = toks.reshape(ntok // 16, 16).T.copy()       # [16, ntok/16]
    wrapped128 = np.tile(wrapped, (8, 1)).astype(np.int16)  # [128, ntok/16]

    w_ih_0T = np.asarray(inputs["w_ih_0"], np.float32).T
    w_hh_0T = np.asarray(inputs["w_hh_0"], np.float32).T
    w_ih_1T = np.asarray(inputs["w_ih_1"], np.float32).T
    w_hh_1T = np.asarray(inputs["w_hh_1"], np.float32).T
    b0sum = (np.asarray(inputs["b_ih_0"], np.float32) +
             np.asarray(inputs["b_hh_0"], np.float32))
    b1sum = (np.asarray(inputs["b_ih_1"], np.float32) +
             np.asarray(inputs["b_hh_1"], np.float32))

    in_maps = []
    for c in range(NCORES):
        m = {"tok": wrapped128,
             "embc": emb[:, 128 * c:128 * (c + 1)].astype(BF16)}

        def tiles(wT, nk, kperm=None):
            """lhsT tiles [(k*4+m)*128, 128]; cols = this core's 512 gates in
            GATE_ORDER; row-chunk k taken from contraction chunk kperm[k]."""
            cols = np.concatenate(
                [wT[:, H * gb + HC * c: H * gb + HC * (c + 1)]
                 for gb in GATE_ORDER], axis=1)  # [K, 512]
            arr = np.zeros((nk * 4 * 128, 128), dtype=BF16)
            for k in range(nk):
                ks = k if kperm is None else kperm[k]
                for mm in range(4):
                    arr[(k * 4 + mm) * 128:(k * 4 + mm + 1) * 128] = \
                        cols[128 * ks:128 * (ks + 1),
                             128 * mm:128 * (mm + 1)].astype(BF16)
            return arr

        slot_perm = [sigma(c, d, d2d_xor) for d in range(NCORES)]
        m["w_p0"] = tiles(w_ih_0T, KE)
        m["w_r0"] = tiles(w_hh_0T, KE, kperm=slot_perm)

        w1 = np.zeros((2 * KE * 4 * 128, 128), dtype=BF16)
        w1[:KE * 4 * 128] = tiles(w_ih_1T, KE, kperm=slot_perm)
        w1[KE * 4 * 128:] = tiles(w_hh_1T, KE, kperm=slot_perm)
        m["w_r1"] = w1

        def bias(bsum):
            v = np.concatenate(
                [bsum[H * gb + HC * c: H * gb + HC * (c + 1)]
                 for gb in GATE_ORDER])
            return v.reshape(4 * 128, 1).astype(np.float32)

        m["b0"] = bias(b0sum)
        m["b1b"] = np.tile(bias(b1sum), (1, B)).astype(np.float32)
        in_maps.append(m)
    return in_maps


def _assemble(results):
    h_n = np.zeros((2, B, H), np.float32)
    c_n = np.zeros((2, B, H), np.float32)
    for c in range(NCORES):
        o = results[c]["out"]
        h_n[0][:, HC * c:HC * (c + 1)] = o[0:128].T
        c_n[0][:, HC * c:HC * (c + 1)] = o[128:256].T
        h_n[1][:, HC * c:HC * (c + 1)] = o[256:384].T
        c_n[1][:, HC * c:HC * (c + 1)] = o[384:512].T
    return h_n, c_n


def run_on_hw(inputs, n_steps=S, trace=False):
    _ensure_axon_hooks()
    from concourse.bass_utils import run_bass_kernel_spmd
    if n_steps not in _CACHE:
        _CACHE[n_steps] = build_nc(n_steps)
    nc = _CACHE[n_steps]
    in_maps = _host_prep(inputs, n_steps, d2d_xor=2)
    res = run_bass_kernel_spmd(nc, in_maps, core_ids=list(range(NCORES)),
                               trace=trace)
    h_n, c_n = _assemble(res.results)
    return (h_n, c_n), res


def kernel(**inputs):
    (h_n, c_n), _ = run_on_hw(inputs, S, trace=False)
    return (h_n, c_n)


if __name__ == "__main__":
    ns = int(os.environ.get("NSTEPS", "4"))
    build_nc(ns)
    print("build OK", ns)


# revision 14
# speedup vs baseline: 2.7650x; 2.5943x over previous
"""Trainium2 Bass kernel for nn_Encoder_5248450035714 (2-layer LSTM encoder).

x = emb[input_seq]; two LSTM layers; returns (h_n, c_n) each [2, B, H].
S=256, B=64, E=H=1024, vocab 32000.

Sharding: tensor-parallel over the 4H gate dim across 8 cores. Core c
owns 128 rows of each gate block (order i, g, f, o) => 512 gate cols =>
h-dims [128c, 128c+128). Per step: weight-stationary bf16 matmuls
produce the transposed gate chunk [512, 64] in PSUM; ACT/DVE apply the
LSTM cell; the h-chunk^T [128, 64] is AllGathered so every core has the
full h^T for the next step. Layer-0 input projection is precomputed as
one big GEMM from the gathered x^T; layer-1's input projection is fused
into the per-step matmul (K = 2048 over [ys0_t; h1]).
"""
import os
import sys

sys.path.insert(0, "/opt/trn_rl_repo")

import numpy as np
import ml_dtypes

BF16 = ml_dtypes.bfloat16

S, B, VOCAB, E, H = 256, 64, 32000, 1024, 1024
NCORES = 8
HC = H // NCORES          # 128 h-dims per core
TOK = S * B               # 16384 tokens
KE = E // 128             # 8 contraction chunks over E/H
GATE_ORDER = (0, 2, 1, 3)  # i, g, f, o (block index into the 4H dim)

_CACHE = {}


def _ensure_axon_hooks():
    try:
        import antenv
        if "/opt/trn_rl_repo/antenv" not in list(antenv.__path__):
            antenv.__path__.append("/opt/trn_rl_repo/antenv")
    except Exception:
        pass


def build_nc(n_steps=S):
    import concourse.bacc as bacc
    import concourse.mybir as mybir
    import concourse.tile as tile

    dt = mybir.dt
    AF = mybir.ActivationFunctionType
    nc = bacc.Bacc("TRN2", target_bir_lowering=False, debug=False,
                   num_devices=NCORES)
    ntok = n_steps * B
    NT = min(512, ntok)   # token tile for proj0

    # ---- per-core inputs (host-sharded) ----
    tok = nc.dram_tensor("tok", [128, ntok // 16], dt.int16,
                         kind="ExternalInput")
    embc = nc.dram_tensor("embc", [VOCAB, 128], dt.bfloat16,
                          kind="ExternalInput")
    w_p0 = nc.dram_tensor("w_p0", [KE * 4 * 128, 128], dt.bfloat16,
                          kind="ExternalInput")   # proj0 lhsT tiles [k][m]
    w_r0 = nc.dram_tensor("w_r0", [KE * 4 * 128, 128], dt.bfloat16,
                          kind="ExternalInput")   # rec0 W_hh0^T tiles
    w_r1 = nc.dram_tensor("w_r1", [2 * KE * 4 * 128, 128], dt.bfloat16,
                          kind="ExternalInput")   # rec1 [W_ih1;W_hh1]^T tiles
    b0 = nc.dram_tensor("b0", [4 * 128, 1], dt.float32, kind="ExternalInput")
    b1 = nc.dram_tensor("b1", [4 * 128, 1], dt.float32, kind="ExternalInput")

    out = nc.dram_tensor("out", [4 * 128, B], dt.float32,
                         kind="ExternalOutput")

    xp0 = nc.dram_tensor("xp0", [4 * 128, ntok], dt.float32, kind="Internal")

    rg = [list(range(NCORES))]

    with tile.TileContext(nc) as tc:
        with tc.tile_pool(name="dram", bufs=1, space="DRAM") as dram, \
             tc.tile_pool(name="wpool", bufs=1) as wpool, \
             tc.tile_pool(name="gather", bufs=1) as gpool, \
             tc.tile_pool(name="xtiles", bufs=2) as xpool, \
             tc.tile_pool(name="psum", bufs=1, space="PSUM") as psum_pool, \
             tc.tile_pool(name="ew", bufs=3) as ewpool, \
             tc.tile_pool(name="state", bufs=1) as spool:


            def dma_blocks(dst2d, src, nblk, f):
                """DRAM [(n p), f] -> SBUF [p, (n f)] block-row layout."""
                nc.sync.dma_start(
                    dst2d.rearrange("p (n f) -> p n f", f=f),
                    src.rearrange("(n p) f -> p n f", p=128))

            # ============ Phase 1: gather x^T e-chunk ============
            idx_sb = gpool.tile([128, ntok // 16], dt.int16, tag="idx")
            nc.sync.dma_start(idx_sb[:], tok[:])
            xt_mine = gpool.tile([128, 1, ntok], dt.bfloat16, tag="xt")
            GCH = min(256, ntok)  # idxs per gather instruction
            for gi in range(ntok // GCH):
                nc.gpsimd.dma_gather(
                    xt_mine[:, :, GCH * gi:GCH * (gi + 1)],
                    embc[:],
                    idx_sb[:, (GCH // 16) * gi:(GCH // 16) * (gi + 1)],
                    num_idxs=GCH, num_idxs_reg=GCH, elem_size=128,
                    transpose=True,
                )

            # ============ Phase 2: AllGather x^T ============
            ag_in = dram.tile([128, ntok], dt.bfloat16, tag="agin")
            xt_full = dram.tile([NCORES * 128, ntok], dt.bfloat16, tag="xtf",
                                addr_space="Shared")
            nc.sync.dma_start(ag_in[:], xt_mine[:, 0, :])
            nc.gpsimd.collective_compute(
                "AllGather", mybir.AluOpType.bypass,
                ins=[ag_in.opt()], outs=[xt_full.opt()], replica_groups=rg,
            )

            # ============ Phase 3: proj0 GEMM ============
            w0_sb = wpool.tile([128, KE * 4 * 128], dt.bfloat16, tag="w0")
            dma_blocks(w0_sb[:], w_p0[:], KE * 4, 128)
            b0_sb = wpool.tile([128, 4], dt.float32, tag="b0")
            dma_blocks(b0_sb[:], b0[:], 4, 1)
            for tt in range(ntok // NT):
                rhs = []
                for k in range(KE):
                    r = xpool.tile([128, NT], dt.bfloat16, tag=f"rhs{k}")
                    nc.sync.dma_start(
                        r[:], xt_full[128 * k:128 * (k + 1),
                                      NT * tt:NT * (tt + 1)])
                    rhs.append(r)
                for m in range(4):
                    ps = psum_pool.tile([128, NT], dt.float32, tag=f"ps{m}")
                    for k in range(KE):
                        nc.tensor.matmul(
                            ps[:],
                            w0_sb[:, (k * 4 + m) * 128:(k * 4 + m + 1) * 128],
                            rhs[k][:],
                            start=(k == 0), stop=(k == KE - 1))
                    xo = ewpool.tile([128, NT], dt.float32, tag="xo")
                    nc.scalar.activation(xo[:], ps[:], AF.Identity,
                                         bias=b0_sb[:, m:m + 1])
                    nc.sync.dma_start(
                        xp0[128 * m:128 * (m + 1), NT * tt:NT * (tt + 1)],
                        xo[:])

            # ============ Phases 4+5: recurrences ============
            w0r_sb = wpool.tile([128, KE * 4 * 128], dt.bfloat16, tag="w0r")
            dma_blocks(w0r_sb[:], w_r0[:], KE * 4, 128)
            w1r_sb = wpool.tile([128, 2 * KE * 4 * 128], dt.bfloat16,
                                tag="w1r")
            dma_blocks(w1r_sb[:], w_r1[:], 2 * KE * 4, 128)
            b1_sb = wpool.tile([128, 4], dt.float32, tag="b1")
            dma_blocks(b1_sb[:], b1[:], 4, 1)

            # persistent state
            c0_sb = spool.tile([128, B], dt.float32, tag="c0")
            c1_sb = spool.tile([128, B], dt.float32, tag="c1")
            nc.vector.memset(c0_sb[:], 0.0)
            nc.vector.memset(c1_sb[:], 0.0)
            # double-buffered combined-gather rhs blocks: 16 AG-order blocks,
            # block 2k = h0 chunk k, block 2k+1 = h1 chunk k
            hx_buf = [spool.tile([128, 2 * KE * B], dt.bfloat16,
                                 tag=f"hxb{i}", name=f"hxb{i}")
                      for i in range(2)]

            agout = [dram.tile([2 * NCORES * 128, B], dt.bfloat16,
                               tag=f"agout{t}", name=f"agout{t}",
                               addr_space="Shared")
                     for t in range(n_steps)]
            agb = dram.tile([2, 2 * 128, B], dt.bfloat16, tag="agb")
            # h1(-1) = 0 rides AG(0)
            zt = spool.tile([128, B], dt.bfloat16, tag="zt")
            nc.vector.memset(zt[:], 0.0)
            nc.sync.dma_start(agb[0, 128:256, :], zt[:])

            sig, tnh = AF.Sigmoid, AF.Tanh

            def lstm_step(layer, t, rhs_of, nk_active, c_sb):
                """One LSTM step. rhs_of(k) -> [128, B] bf16 AP; matmuls
                run over k < nk_active. Returns h_new bf16 [128, B]."""
                w_sb = w0r_sb if layer == 0 else w1r_sb
                ps = psum_pool.tile([128, 4 * B], dt.float32,
                                    tag=f"gps{layer}{t % 2}")
                if layer == 0:
                    xp_sb = ewpool.tile([128, 4 * B], dt.float32,
                                        tag=f"xp{t % 2}")
                    dma_blocks(xp_sb[:], xp0[:, B * t:B * (t + 1)], 4, B)
                for m in range(4):
                    for k in range(nk_active):
                        nc.tensor.matmul(
                            ps[:, B * m:B * (m + 1)],
                            w_sb[:, (k * 4 + m) * 128:(k * 4 + m + 1) * 128],
                            rhs_of(k),
                            start=(k == 0), stop=(k == nk_active - 1))
                g_sb = ewpool.tile([128, 4 * B], dt.float32, tag=f"g{layer}")
                for m, fn in ((0, sig), (1, tnh), (2, sig), (3, sig)):
                    sl = slice(B * m, B * (m + 1))
                    if nk_active == 0:
                        # t==0 layer0: gates = xp only (bias folded in)
                        nc.scalar.activation(g_sb[:, sl], xp_sb[:, sl], fn)
                    elif layer == 0:
                        nc.vector.tensor_add(g_sb[:, sl], ps[:, sl],
                                             xp_sb[:, sl])
                        nc.scalar.activation(g_sb[:, sl], g_sb[:, sl], fn)
                    else:
                        nc.scalar.activation(g_sb[:, sl], ps[:, sl], fn,
                                             bias=b1_sb[:, m:m + 1])
                ig = ewpool.tile([128, B], dt.float32, tag=f"ig{layer}")
                nc.vector.tensor_mul(ig[:], g_sb[:, 0:B], g_sb[:, B:2 * B])
                fc = ewpool.tile([128, B], dt.float32, tag=f"fc{layer}")
                nc.vector.tensor_mul(fc[:], g_sb[:, 2 * B:3 * B], c_sb[:])
                nc.vector.tensor_add(c_sb[:], ig[:], fc[:])
                tc_sb = ewpool.tile([128, B], dt.float32, tag=f"tc{layer}")
                nc.scalar.activation(tc_sb[:], c_sb[:], tnh)
                h_new = ewpool.tile([128, B], dt.bfloat16,
                                    tag=f"hn{layer}{t % 2}")
                nc.vector.tensor_mul(h_new[:], g_sb[:, 3 * B:4 * B],
                                     tc_sb[:])
                return h_new

            # ---- combined recurrence: one AllGather per step carrying
            # [h0(tau) | h1(tau-1)] — halves the serialized collective count.
            # AG-order blocks of hx_buf: block 2k = h0 chunk k (ys0),
            # block 2k+1 = h1 chunk k.
            for tau in range(n_steps + 1):
                if tau < n_steps:
                    hb = hx_buf[(tau - 1) % 2]
                    h0c = lstm_step(0, tau,
                                    lambda k: hb[:, B * 2 * k:B * (2 * k + 1)],
                                    0 if tau == 0 else KE, c0_sb)
                    nc.sync.dma_start(agb[tau % 2, 0:128, :], h0c[:])
                    if tau == n_steps - 1:
                        h0_final = h0c
                if tau >= 1:
                    t1 = tau - 1
                    hxb = hx_buf[t1 % 2]

                    def rhs1(k, hxb=hxb):
                        if k < KE:
                            return hxb[:, B * 2 * k:B * (2 * k + 1)]
                        kk = k - KE
                        return hxb[:, B * (2 * kk + 1):B * (2 * kk + 2)]

                    h1c = lstm_step(1, t1, rhs1,
                                    KE if t1 == 0 else 2 * KE, c1_sb)
                    if t1 < n_steps - 1:
                        nc.sync.dma_start(agb[tau % 2, 128:256, :], h1c[:])
                    else:
                        h1_final = h1c
                if tau < n_steps:
                    nc.gpsimd.collective_compute(
                        "AllGather", mybir.AluOpType.bypass,
                        ins=[agb[tau % 2, :, :].opt()],
                        outs=[agout[tau].opt()], replica_groups=rg)
                    dma_blocks(hx_buf[tau % 2][:], agout[tau][:],
                               2 * KE, B)

            # ---- outputs ----
            of = ewpool.tile([128, B], dt.float32, tag="of")
            nc.scalar.activation(of[:], h0_final[:], AF.Copy)
            nc.sync.dma_start(out[0:128, :], of[:])
            nc.sync.dma_start(out[128:256, :], c0_sb[:])
            of2 = ewpool.tile([128, B], dt.float32, tag="of2")
            nc.scalar.activation(of2[:], h1_final[:], AF.Copy)
            nc.sync.dma_start(out[256:384, :], of2[:])
            nc.sync.dma_start(out[384:512, :], c1_sb[:])

    nc.compile()
    return nc


def _host_prep(inputs, n_steps=S):
    """Build per-core in_maps from full inputs."""
    seq = np.asarray(inputs["input_seq"])[:n_steps].astype(np.int64)
    emb = np.asarray(inputs["emb"], dtype=np.float32)
    ntok = n_steps * B

    toks = seq.reshape(-1).astype(np.int16)  # vocab < 32768
    wrapped = toks.reshape(ntok // 16, 16).T.copy()       # [16, ntok/16]
    wrapped128 = np.tile(wrapped, (8, 1)).astype(np.int16)  # [128, ntok/16]

    w_ih_0T = np.asarray(inputs["w_ih_0"], np.float32).T
    w_hh_0T = np.asarray(inputs["w_hh_0"], np.float32).T
    w1T = np.concatenate([np.asarray(inputs["w_ih_1"], np.float32).T,
                          np.asarray(inputs["w_hh_1"], np.float32).T], axis=0)
    b0sum = (np.asarray(inputs["b_ih_0"], np.float32) +
             np.asarray(inputs["b_hh_0"], np.float32))
    b1sum = (np.asarray(inputs["b_ih_1"], np.float32) +
             np.asarray(inputs["b_hh_1"], np.float32))

    in_maps = []
    for c in range(NCORES):
        m = {"tok": wrapped128,
             "embc": emb[:, 128 * c:128 * (c + 1)].astype(BF16)}

        def tiles(wT, nk):
            cols = np.concatenate(
                [wT[:, H * gb + HC * c: H * gb + HC * (c + 1)]
                 for gb in GATE_ORDER], axis=1)  # [K, 512]
            arr = np.zeros((nk * 4 * 128, 128), dtype=BF16)
            for k in range(nk):
                for mm in range(4):
                    arr[(k * 4 + mm) * 128:(k * 4 + mm + 1) * 128] = \
                        cols[128 * k:128 * (k + 1),
                             128 * mm:128 * (mm + 1)].astype(BF16)
            return arr

        m["w_p0"] = tiles(w_ih_0T, KE)
        m["w_r0"] = tiles(w_hh_0T, KE)
        m["w_r1"] = tiles(w1T, 2 * KE)

        def bias(bsum):
            v = np.concatenate(
                [bsum[H * gb + HC * c: H * gb + HC * (c + 1)]
                 for gb in GATE_ORDER])
            return v.reshape(4 * 128, 1).astype(np.float32)

        m["b0"] = bias(b0sum)
        m["b1"] = bias(b1sum)
        in_maps.append(m)
    return in_maps


def _assemble(results):
    h_n = np.zeros((2, B, H), np.float32)
    c_n = np.zeros((2, B, H), np.float32)
    for c in range(NCORES):
        o = results[c]["out"]
        h_n[0][:, HC * c:HC * (c + 1)] = o[0:128].T
        c_n[0][:, HC * c:HC * (c + 1)] = o[128:256].T
        h_n[1][:, HC * c:HC * (c + 1)] = o[256:384].T
        c_n[1][:, HC * c:HC * (c + 1)] = o[384:512].T
    return h_n, c_n


def run_on_hw(inputs, n_steps=S, trace=False):
    _ensure_axon_hooks()
    from concourse.bass_utils import run_bass_kernel_spmd
    if n_steps not in _CACHE:
        _CACHE[n_steps] = build_nc(n_steps)
    nc = _CACHE[n_steps]
    in_maps = _host_prep(inputs, n_steps)
    res = run_bass_kernel_spmd(nc, in_maps, core_ids=list(range(NCORES)),
                               trace=trace)
    h_n, c_n = _assemble(res.results)
    return (h_n, c_n), res


def kernel(**inputs):
    (h_n, c_n), _ = run_on_hw(inputs, S, trace=False)
    return (h_n, c_n)


if __name__ == "__main__":
    ns = int(os.environ.get("NSTEPS", "4"))
    build_nc(ns)
    print("build OK", ns)



# revision 20
# speedup vs baseline: 3.1656x; 1.1449x over previous
"""Trainium2 Bass kernel for nn_Encoder_5248450035714 (2-layer LSTM encoder).

x = emb[input_seq]; two LSTM layers; returns (h_n, c_n) each [2, B, H].
S=256, B=64, E=H=1024, vocab 32000.

Sharding: tensor-parallel over the 4H gate dim across 8 cores. Core c
owns 128 rows of each gate block (order i, g, f, o) => 512 gate cols =>
h-dims [128c, 128c+128). Per step: weight-stationary bf16 matmuls
produce the transposed gate chunk [512, 64] in PSUM; ACT/DVE apply the
LSTM cell; the h-chunk^T [128, 64] is AllGathered so every core has the
full h^T for the next step. Layer-0 input projection is precomputed as
one big GEMM from the gathered x^T; layer-1's input projection is fused
into the per-step matmul (K = 2048 over [ys0_t; h1]).
"""
import os
import sys

sys.path.insert(0, "/opt/trn_rl_repo")

import numpy as np
import ml_dtypes

BF16 = ml_dtypes.bfloat16

S, B, VOCAB, E, H = 256, 64, 32000, 1024, 1024
NCORES = 8
HC = H // NCORES          # 128 h-dims per core
TOK = S * B               # 16384 tokens
KE = E // 128             # 8 contraction chunks over E/H
GATE_ORDER = (0, 1, 3, 2)  # i, f, o, g (block index into the 4H dim)

_CACHE = {}


def _ensure_axon_hooks():
    try:
        import antenv
        if "/opt/trn_rl_repo/antenv" not in list(antenv.__path__):
            antenv.__path__.append("/opt/trn_rl_repo/antenv")
    except Exception:
        pass


def build_nc(n_steps=S):
    import concourse.bacc as bacc
    import concourse.mybir as mybir
    import concourse.tile as tile

    dt = mybir.dt
    AF = mybir.ActivationFunctionType
    nc = bacc.Bacc("TRN2", target_bir_lowering=False, debug=False,
                   num_devices=NCORES)
    ntok = n_steps * B
    NT = min(512, ntok)   # token tile for proj0

    # ---- per-core inputs (host-sharded) ----
    tok = nc.dram_tensor("tok", [128, ntok // 16], dt.int16,
                         kind="ExternalInput")
    embc = nc.dram_tensor("embc", [VOCAB, 128], dt.bfloat16,
                          kind="ExternalInput")
    w_p0 = nc.dram_tensor("w_p0", [KE * 4 * 128, 128], dt.bfloat16,
                          kind="ExternalInput")   # proj0 lhsT tiles [k][m]
    w_r0 = nc.dram_tensor("w_r0", [KE * 4 * 128, 128], dt.bfloat16,
                          kind="ExternalInput")   # rec0 W_hh0^T tiles
    w_r1 = nc.dram_tensor("w_r1", [2 * KE * 4 * 128, 128], dt.bfloat16,
                          kind="ExternalInput")   # rec1 [W_ih1;W_hh1]^T tiles
    b0 = nc.dram_tensor("b0", [4 * 128, 1], dt.float32, kind="ExternalInput")
    b1b = nc.dram_tensor("b1b", [4 * 128, B], dt.float32,
                         kind="ExternalInput")  # b1 broadcast over batch

    out = nc.dram_tensor("out", [4 * 128, B], dt.float32,
                         kind="ExternalOutput")

    xp0 = nc.dram_tensor("xp0", [4 * 128, ntok], dt.float32, kind="Internal")

    rg = [list(range(NCORES))]

    with tile.TileContext(nc) as tc:
        with tc.tile_pool(name="dram", bufs=1, space="DRAM") as dram, \
             tc.tile_pool(name="wpool", bufs=1) as wpool, \
             tc.tile_pool(name="gather", bufs=1) as gpool, \
             tc.tile_pool(name="xtiles", bufs=2) as xpool, \
             tc.tile_pool(name="psum", bufs=1, space="PSUM") as psum_pool, \
             tc.tile_pool(name="ew", bufs=3) as ewpool, \
             tc.tile_pool(name="state", bufs=1) as spool:


            def dma_blocks(dst2d, src, nblk, f):
                """DRAM [(n p), f] -> SBUF [p, (n f)] block-row layout."""
                nc.sync.dma_start(
                    dst2d.rearrange("p (n f) -> p n f", f=f),
                    src.rearrange("(n p) f -> p n f", p=128))

            # ============ Phase 1: gather x^T e-chunk ============
            idx_sb = gpool.tile([128, ntok // 16], dt.int16, tag="idx")
            nc.sync.dma_start(idx_sb[:], tok[:])
            xt_mine = gpool.tile([128, 1, ntok], dt.bfloat16, tag="xt")
            GCH = min(256, ntok)  # idxs per gather instruction
            for gi in range(ntok // GCH):
                nc.gpsimd.dma_gather(
                    xt_mine[:, :, GCH * gi:GCH * (gi + 1)],
                    embc[:],
                    idx_sb[:, (GCH // 16) * gi:(GCH // 16) * (gi + 1)],
                    num_idxs=GCH, num_idxs_reg=GCH, elem_size=128,
                    transpose=True,
                )

            # ============ Phase 2: AllGather x^T ============
            ag_in = dram.tile([128, ntok], dt.bfloat16, tag="agin")
            xt_full = dram.tile([NCORES * 128, ntok], dt.bfloat16, tag="xtf",
                                addr_space="Shared")
            nc.sync.dma_start(ag_in[:], xt_mine[:, 0, :])
            nc.gpsimd.collective_compute(
                "AllGather", mybir.AluOpType.bypass,
                ins=[ag_in.opt()], outs=[xt_full.opt()], replica_groups=rg,
            )

            # ============ Phase 3: proj0 GEMM ============
            w0_sb = wpool.tile([128, KE * 4 * 128], dt.bfloat16, tag="w0")
            dma_blocks(w0_sb[:], w_p0[:], KE * 4, 128)
            b0_sb = wpool.tile([128, 4], dt.float32, tag="b0")
            dma_blocks(b0_sb[:], b0[:], 4, 1)
            for tt in range(ntok // NT):
                rhs = []
                for k in range(KE):
                    r = xpool.tile([128, NT], dt.bfloat16, tag=f"rhs{k}")
                    nc.sync.dma_start(
                        r[:], xt_full[128 * k:128 * (k + 1),
                                      NT * tt:NT * (tt + 1)])
                    rhs.append(r)
                for m in range(4):
                    ps = psum_pool.tile([128, NT], dt.float32, tag=f"ps{m}")
                    for k in range(KE):
                        nc.tensor.matmul(
                            ps[:],
                            w0_sb[:, (k * 4 + m) * 128:(k * 4 + m + 1) * 128],
                            rhs[k][:],
                            start=(k == 0), stop=(k == KE - 1))
                    xo = ewpool.tile([128, NT], dt.float32, tag="xo")
                    nc.scalar.activation(xo[:], ps[:], AF.Identity,
                                         bias=b0_sb[:, m:m + 1])
                    nc.sync.dma_start(
                        xp0[128 * m:128 * (m + 1), NT * tt:NT * (tt + 1)],
                        xo[:])

            # ============ Phases 4+5: recurrences ============
            w0r_sb = wpool.tile([128, KE * 4 * 128], dt.bfloat16, tag="w0r")
            dma_blocks(w0r_sb[:], w_r0[:], KE * 4, 128)
            w1r_sb = wpool.tile([128, 2 * KE * 4 * 128], dt.bfloat16,
                                tag="w1r")
            dma_blocks(w1r_sb[:], w_r1[:], 2 * KE * 4, 128)
            b1_sb = wpool.tile([128, 4 * B], dt.float32, tag="b1")
            dma_blocks(b1_sb[:], b1b[:], 4, B)

            # persistent state
            c0_sb = spool.tile([128, B], dt.float32, tag="c0")
            c1_sb = spool.tile([128, B], dt.float32, tag="c1")
            nc.vector.memset(c0_sb[:], 0.0)
            nc.vector.memset(c1_sb[:], 0.0)
            # double-buffered rhs blocks
            h0_buf = [spool.tile([128, KE * B], dt.bfloat16,
                                 tag=f"h0b{i}", name=f"h0b{i}")
                      for i in range(2)]
            h1_buf = [spool.tile([128, KE * B], dt.bfloat16,
                                 tag=f"h1b{i}", name=f"h1b{i}")
                      for i in range(2)]

            ys0 = [dram.tile([NCORES * 128, B], dt.bfloat16,
                              tag=f"ys0_{t}", name=f"ys0_{t}",
                              addr_space="Shared")
                   for t in range(n_steps)]
            h1ag = [dram.tile([NCORES * 128, B], dt.bfloat16,
                              tag=f"h1ag_{t}", name=f"h1ag_{t}",
                              addr_space="Shared")
                    for t in range(max(n_steps - 1, 1))]
            agb0 = dram.tile([2, 128, B], dt.bfloat16, tag="agb0")
            agb1 = dram.tile([2, 128, B], dt.bfloat16, tag="agb1")

            sig, tnh = AF.Sigmoid, AF.Tanh

            def lstm_step(layer, t, rhs_of, nk_active, c_sb):
                """One LSTM step. rhs_of(k) -> [128, B] bf16 AP; matmuls
                run over k < nk_active. Returns h_new bf16 [128, B]."""
                w_sb = w0r_sb if layer == 0 else w1r_sb
                ps = psum_pool.tile([128, 4 * B], dt.float32,
                                    tag=f"gps{layer}{t % 2}")
                if layer == 0:
                    xp_sb = ewpool.tile([128, 4 * B], dt.float32,
                                        tag=f"xp{t % 2}")
                    dma_blocks(xp_sb[:], xp0[:, B * t:B * (t + 1)], 4, B)
                for m in range(4):
                    for k in range(nk_active):
                        nc.tensor.matmul(
                            ps[:, B * m:B * (m + 1)],
                            w_sb[:, (k * 4 + m) * 128:(k * 4 + m + 1) * 128],
                            rhs_of(k),
                            start=(k == 0), stop=(k == nk_active - 1))
                # gate cols: [i f o | g] — one wide sigmoid + one tanh
                B3 = 3 * B
                g_sb = ewpool.tile([128, 4 * B], dt.float32, tag=f"g{layer}")
                if nk_active == 0:
                    # t==0 layer0: gates = xp only (bias folded in)
                    nc.scalar.activation(g_sb[:, 0:B3], xp_sb[:, 0:B3], sig)
                    nc.scalar.activation(g_sb[:, B3:], xp_sb[:, B3:], tnh)
                else:
                    addend = xp_sb if layer == 0 else b1_sb
                    nc.vector.tensor_add(g_sb[:], ps[:], addend[:])
                    nc.scalar.activation(g_sb[:, 0:B3], g_sb[:, 0:B3], sig)
                    nc.scalar.activation(g_sb[:, B3:], g_sb[:, B3:], tnh)
                ig = ewpool.tile([128, B], dt.float32, tag=f"ig{layer}")
                nc.vector.tensor_mul(ig[:], g_sb[:, 0:B], g_sb[:, 3 * B:])
                fc = ewpool.tile([128, B], dt.float32, tag=f"fc{layer}")
                nc.vector.tensor_mul(fc[:], g_sb[:, B:2 * B], c_sb[:])
                nc.vector.tensor_add(c_sb[:], ig[:], fc[:])
                tc_sb = ewpool.tile([128, B], dt.float32, tag=f"tc{layer}")
                nc.scalar.activation(tc_sb[:], c_sb[:], tnh)
                h_new = ewpool.tile([128, B], dt.bfloat16,
                                    tag=f"hn{layer}{t % 2}")
                nc.vector.tensor_mul(h_new[:], g_sb[:, 2 * B:3 * B],
                                     tc_sb[:])
                return h_new

            # ---- interleaved rec0 (step tau) + rec1 (step tau-1) ----
            # AG0(tau) flight overlaps rec1 step tau-1's compute; AG1(tau-1)
            # flight overlaps rec0 step tau+1's compute.
            for tau in range(n_steps + 1):
                if tau < n_steps:
                    hb = h0_buf[tau % 2]
                    h0c = lstm_step(0, tau,
                                    lambda k: hb[:, B * k:B * (k + 1)],
                                    0 if tau == 0 else KE, c0_sb)
                    nc.sync.dma_start(agb0[tau % 2, :, :], h0c[:])
                    nc.gpsimd.collective_compute(
                        "AllGather", mybir.AluOpType.bypass,
                        ins=[agb0[tau % 2, :, :].opt()],
                        outs=[ys0[tau].opt()], replica_groups=rg)
                    dma_blocks(h0_buf[(tau + 1) % 2][:], ys0[tau][:],
                               KE, B)
                    if tau == n_steps - 1:
                        h0_final = h0c
                if tau >= 1:
                    t1 = tau - 1
                    hxb = h0_buf[(t1 + 1) % 2]
                    hhb = h1_buf[t1 % 2]

                    def rhs1(k, hxb=hxb, hhb=hhb):
                        if k < KE:
                            return hxb[:, B * k:B * (k + 1)]
                        return hhb[:, B * (k - KE):B * (k - KE + 1)]

                    h1c = lstm_step(1, t1, rhs1,
                                    KE if t1 == 0 else 2 * KE, c1_sb)
                    if t1 < n_steps - 1:
                        nc.sync.dma_start(agb1[t1 % 2, :, :], h1c[:])
                        nc.gpsimd.collective_compute(
                            "AllGather", mybir.AluOpType.bypass,
                            ins=[agb1[t1 % 2, :, :].opt()],
                            outs=[h1ag[t1].opt()], replica_groups=rg)
                        dma_blocks(h1_buf[(t1 + 1) % 2][:], h1ag[t1][:],
                                   KE, B)
                    else:
                        h1_final = h1c

            # ---- outputs ----
            of = ewpool.tile([128, B], dt.float32, tag="of")
            nc.scalar.activation(of[:], h0_final[:], AF.Copy)
            nc.sync.dma_start(out[0:128, :], of[:])
            nc.sync.dma_start(out[128:256, :], c0_sb[:])
            of2 = ewpool.tile([128, B], dt.float32, tag="of2")
            nc.scalar.activation(of2[:], h1_final[:], AF.Copy)
            nc.sync.dma_start(out[256:384, :], of2[:])
            nc.sync.dma_start(out[384:512, :], c1_sb[:])

    nc.compile()
    return nc


def _host_prep(inputs, n_steps=S):
    """Build per-core in_maps from full inputs."""
    seq = np.asarray(inputs["input_seq"])[:n_steps].astype(np.int64)
    emb = np.asarray(inputs["emb"], dtype=np.float32)
    ntok = n_steps * B

    toks = seq.reshape(-1).astype(np.int16)  # vocab < 32768
    wrapped = toks.reshape(ntok // 16, 16).T.copy()       # [16, ntok/16]
    wrapped128 = np.tile(wrapped, (8, 1)).astype(np.int16)  # [128, ntok/16]

    w_ih_0T = np.asarray(inputs["w_ih_0"], np.float32).T
    w_hh_0T = np.asarray(inputs["w_hh_0"], np.float32).T
    w1T = np.concatenate([np.asarray(inputs["w_ih_1"], np.float32).T,
                          np.asarray(inputs["w_hh_1"], np.float32).T], axis=0)
    b0sum = (np.asarray(inputs["b_ih_0"], np.float32) +
             np.asarray(inputs["b_hh_0"], np.float32))
    b1sum = (np.asarray(inputs["b_ih_1"], np.float32) +
             np.asarray(inputs["b_hh_1"], np.float32))

    in_maps = []
    for c in range(NCORES):
        m = {"tok": wrapped128,
             "embc": emb[:, 128 * c:128 * (c + 1)].astype(BF16)}

        def tiles(wT, nk):
            cols = np.concatenate(
                [wT[:, H * gb + HC * c: H * gb + HC * (c + 1)]
                 for gb in GATE_ORDER], axis=1)  # [K, 512]
            arr = np.zeros((nk * 4 * 128, 128), dtype=BF16)
            for k in range(nk):
                for mm in range(4):
                    arr[(k * 4 + mm) * 128:(k * 4 + mm + 1) * 128] = \
                        cols[128 * k:128 * (k + 1),
                             128 * mm:128 * (mm + 1)].astype(BF16)
            return arr

        m["w_p0"] = tiles(w_ih_0T, KE)
        m["w_r0"] = tiles(w_hh_0T, KE)
        m["w_r1"] = tiles(w1T, 2 * KE)

        def bias(bsum):
            v = np.concatenate(
                [bsum[H * gb + HC * c: H * gb + HC * (c + 1)]
                 for gb in GATE_ORDER])
            return v.reshape(4 * 128, 1).astype(np.float32)

        m["b0"] = bias(b0sum)
        m["b1b"] = np.tile(bias(b1sum), (1, B)).astype(np.float32)
        in_maps.append(m)
    return in_maps


def _assemble(results):
    h_n = np.zeros((2, B, H), np.float32)
    c_n = np.zeros((2, B, H), np.float32)
    for c in range(NCORES):
        o = results[c]["out"]
        h_n[0][:, HC * c:HC * (c + 1)] = o[0:128].T
        c_n[0][:, HC * c:HC * (c + 1)] = o[128:256].T
        h_n[1][:, HC * c:HC * (c + 1)] = o[256:384].T
        c_n[1][:, HC * c:HC * (c + 1)] = o[384:512].T
    return h_n, c_n


def run_on_hw(inputs, n_steps=S, trace=False):
    _ensure_axon_hooks()
    from concourse.bass_utils import run_bass_kernel_spmd
    if n_steps not in _CACHE:
        _CACHE[n_steps] = build_nc(n_steps)
    nc = _CACHE[n_steps]
    in_maps = _host_prep(inputs, n_steps)
    res = run_bass_kernel_spmd(nc, in_maps, core_ids=list(range(NCORES)),
                               trace=trace)
    h_n, c_n = _assemble(res.results)
    return (h_n, c_n), res


def kernel(**inputs):
    (h_n, c_n), _ = run_on_hw(inputs, S, trace=False)
    return (h_n, c_n)


if __name__ == "__main__":
    ns = int(os.environ.get("NSTEPS", "4"))
    build_nc(ns)
    print("build OK", ns)

